# revision 2
# baseline (speedup 1.0000x reference)
"""Bahdanau-attention kernel for trn2, data-parallel over batch across 8 cores.

Per-core computation (B_LOC = 4 batches, S = 4096, H = E = 256):
  energy = tanh(hidden @ Wh.T + enc @ We.T + b_attn)      [b, s, e]
  scores = energy . v                                      [b, s]
  attn   = softmax(scores) over s  (no max-subtraction: scores bounded by ||v||_1)
  out    = sum_s attn * enc                                [b, h]

Design (v2 — PE-side transposes, fp8 DoubleRow energy):
  - enc slice read from HBM once via SWDGE cast-DMA (f32 -> bf16), 4
    consecutive s-rows per partition (16KB-contiguous read descriptors).
    The read stream is the ONLY bulk DMA: measured SDMA system throughput
    is ~460 GB/s summed over both sides of every transfer, so the v1 xbar
    transposes (8 MiB rd + 8 MiB wr) cost ~40us of SDMA time that could
    not overlap the reads. They are gone.
  - X^T for the energy matmul is built on the PE instead: 32 per-group
    [128,128] is_transpose matmuls into PSUM (bf16), drained to SBUF as
    fp8e4 by DVE/ACT copy-casts (the cast rides the mandatory PSUM->SBUF
    copy for free).
  - Energy matmul runs in fp8 DoubleRow mode: the full 256-deep h
    contraction in ONE matmul per (eh, b) — lhsT = We^T fp8 [128,2,128]
    (pre-scaled x32 so e4m3 sees ~unit values), rhs = X^T fp8 [128,2,512].
    The x32 is undone by the tanh ACTIVATE's scale=1/32 (bias folds
    qb[e] = hidden @ Wh.T + b_attn as before).
  - v-dot on the PE with v stationary, 4 batches' score strips packed in
    one PSUM bank at partitions {0,32,64,96} (tile_position col packing);
    one Exp per group produces exp(scores)+denominators; exp strips are
    PE-transposed to [s-part, b]; context accumulates in two [2, 512]
    PSUM tiles against the resident native bf16 X (full precision path).
  - softmax normalization (divide by denominator) happens on the host.
  - DMA surgery (much smaller than v1): group reads keep only their
    GpSimd FIFO/ring ordering (the Tile scheduler's exclusive-DMA-device
    sem chains would add a ~2us completion-receipt bubble per read);
    end-block rendezvous waits are deduped.
"""

import numpy as np

B, S, H = 32, 4096, 256
NCORES = 8
BL = B // NCORES  # batches per core
NG = 8            # s-groups of 512 rows
E = H
WSCALE = 32.0     # fp8 pre-scale on We^T, undone in the tanh activation

_CACHE = {}


def _split_multiwait(nc, mybir):
    """This walrus/ISA build allows ONE sync-wait slot per instruction.
    Move extra waits onto same-engine NoOps inserted just before."""
    for blk in nc.m.functions[0].blocks:
        insts = blk.instructions
        out = []
        changed = False
        for inst in insts:
            si = inst.sync_info
            waits = list(si.on_wait) if si is not None else []
            if len(waits) > 1:
                for w in waits[:-1]:
                    nop = mybir.InstNoOp(
                        name=nc.get_next_instruction_name(), ins=[], outs=[]
                    )
                    nop.engine = inst.engine
                    nop.sync_info = mybir.SyncInfo(on_wait=[w], on_update=[])
                    out.append(nop)
                inst.sync_info = mybir.SyncInfo(
                    on_wait=[waits[-1]], on_update=list(si.on_update)
                )
                changed = True
            out.append(inst)
        if changed:
            insts[:] = out


def _dma_surgery(nc, mybir, read_names, setup_read_names=(), verbose=False):
    """Strip the scheduler's false serialization between successive SWDGE
    HBM reads.

    Safety argument: reads write fresh x_res tiles (no reuse) and execute
    in unchanged FIFO order on their own queue, so every DMA lane
    semaphore still reaches each value in the same order as the legacy
    schedule — only earlier. All remaining waits are sem-ge, hence
    monotone-safe.
    """
    blocks = nc.m.functions[0].blocks
    insts = {}
    for blk in blocks:
        for i in blk.instructions:
            insts[i.name] = i

    def is_lane(w):
        return w.ant_name.startswith("DMASW") or w.ant_name.startswith("DMAHW")

    # 1. Reads wait on nothing DMA-related (GpSimd FIFO + SWDGE ring throttle).
    for rn in list(read_names) + list(setup_read_names):
        i = insts[rn]
        si = i.sync_info
        if si is None:
            continue
        keep = [w for w in si.on_wait if not is_lane(w)]
        if verbose and len(keep) != len(si.on_wait):
            print(f"  read {rn}: dropped {len(si.on_wait) - len(keep)} lane waits")
        i.sync_info = mybir.SyncInfo(on_wait=keep, on_update=list(si.on_update))

    # 2. End-block rendezvous: tile makes EVERY engine wait on EVERY
    #    semaphore (engine counters + all 16 DMA lanes) via chains of
    #    single-wait NoOps before the common gather/release barrier. One
    #    waiter per (sem, value) suffices - the barrier then propagates
    #    completion to all engines. Dedupe to shorten the tail.
    for blk in blocks:
        if not (blk.name.startswith("tile_context") and blk.name.endswith("_end")):
            continue
        seen = set()
        for i in blk.instructions:
            si = i.sync_info
            if si is None or not si.on_wait:
                continue
            keep = []
            for w in si.on_wait:
                if w.wait_mode != "sem-ge-imm" or "barrier" in w.ant_name:
                    keep.append(w)
                    continue
                key = (w.ant_name, w.wait_value)
                if key not in seen:
                    seen.add(key)
                    keep.append(w)
            if len(keep) != len(si.on_wait):
                if verbose:
                    print(
                        f"  end {type(i).__name__} {i.name}: "
                        f"{len(si.on_wait)} -> {len(keep)} waits"
                    )
                i.sync_info = mybir.SyncInfo(
                    on_wait=keep, on_update=list(si.on_update)
                )


def _build(verbose=False):
    import concourse.bass as bass
    import concourse.tile as tile
    from concourse import mybir
    from concourse.masks import make_identity

    f32 = mybir.dt.float32
    bf16 = mybir.dt.bfloat16
    fp8 = mybir.dt.float8e4
    AF = mybir.ActivationFunctionType
    DR = mybir.MatmulPerfMode.DoubleRow

    nc = bass.Bass(num_swdge_queues=2, dynamic_dma_scratch_size=65536)
    hid_t = nc.dram_tensor("hidden", [BL, H], f32, kind="ExternalInput")
    enc_t = nc.dram_tensor("enc", [S, BL, H], f32, kind="ExternalInput")
    wat_t = nc.dram_tensor("w_attn", [H, 2 * H], f32, kind="ExternalInput")
    bat_t = nc.dram_tensor("b_attn", [H], f32, kind="ExternalInput")
    wv_t = nc.dram_tensor("w_v", [1, H], f32, kind="ExternalInput")
    # unnormalized context halves + denominators; normalized on host
    ctxu_t = nc.dram_tensor("ctxu", [2, 2, 512], f32, kind="ExternalOutput")
    den_t = nc.dram_tensor("den", [97, 1], f32, kind="ExternalOutput")

    hid = hid_t.ap()
    enc = enc_t.ap()
    wat = wat_t.ap()
    bat = bat_t.ap().rearrange("(o c) -> o c", o=1)  # [1, 256]
    wv = wv_t.ap()

    read_names = []        # instruction names of the SWDGE x_res reads
    setup_read_names = []  # HWDGE setup reads (strip serial-chain waits only)

    with tile.TileContext(nc) as tc:
        with (
            tc.tile_pool(name="const", bufs=1) as cp,
            tc.tile_pool(name="xres", bufs=1) as xrp,
            tc.tile_pool(name="x8", bufs=1) as x8p,
            tc.tile_pool(name="thp", bufs=8) as thp,
            tc.tile_pool(name="stat", bufs=1) as stp,
            tc.tile_pool(name="misc", bufs=2) as wp,
            tc.tile_pool(name="pt", bufs=2, space="PSUM") as ppt,
            tc.tile_pool(name="pe", bufs=2, space="PSUM") as ppe,
            tc.tile_pool(name="ps", bufs=2, space="PSUM") as pps,
            tc.tile_pool(name="pc", bufs=1, space="PSUM") as ppc,
        ):
            # ---------- resident enc: bf16 cast-DMA reads (issued FIRST so
            # the GpSimd engine starts the HBM stream before any setup) ----
            x_res = []
            for g in range(NG):
                t = xrp.tile([128, 4, 4 * H], bf16, tag=f"xr{g}", name=f"xr{g}")
                # 4 consecutive s-rows per partition: each read descriptor
                # covers 16KB contiguous DRAM. softmax and context contract
                # over s, so the s-relabeling is free.
                src = enc[g * 512 : (g + 1) * 512, :, :].rearrange(
                    "(p i) b h -> p i (b h)", i=4
                )
                if g == 0 or g == NG - 1:
                    # 1-MiB halves split by batch pair: head half feeds the
                    # first transpose stage sooner; tail half shortens the
                    # last-group critical chain.
                    r0 = nc.gpsimd.dma_start(out=t[:, :, 0:512], in_=src[:, :, 0:512])
                    r1 = nc.gpsimd.dma_start(out=t[:, :, 512:1024], in_=src[:, :, 512:1024])
                    read_names += [r0.ins.name, r1.ins.name]
                else:
                    r = nc.gpsimd.dma_start(out=t, in_=src)
                    read_names.append(r.ins.name)
                x_res.append(t)

            st_g = [
                stp.tile([97, 512], bf16, tag=f"st{g}", name=f"st{g}")
                for g in range(NG)
            ]
            for g in range(NG):
                nc.vector.memset(st_g[g], 0.0)

            ident = cp.tile([128, 128], f32)
            make_identity(nc, ident)
            ident16 = cp.tile([128, 128], bf16)
            nc.vector.tensor_copy(out=ident16, in_=ident)

            # fp8 X^T per group: [128 h, (b, hh, i, 128 s)]
            xt8 = [
                x8p.tile([128, 4096], fp8, tag=f"x8{g}", name=f"x8{g}").rearrange(
                    "p (b hh i c) -> p b hh i c", b=4, hh=2, c=128
                )
                for g in range(NG)
            ]

            u_g = [
                stp.tile([128, BL, 4], bf16, tag=f"ug{g}", name=f"ug{g}")
                for g in range(NG)
            ]
            acc_all = stp.tile([97, NG], f32)
            wet8 = cp.tile([128, 2, 2 * E], fp8, tag="wet8", name="wet8")
            qb = [cp.tile([128, BL], f32, tag=f"qb{i}", name=f"qb{i}") for i in range(2)]
            vt16 = [cp.tile([128, 1], bf16, tag=f"vt{i}", name=f"vt{i}") for i in range(2)]

            # ---------------- setup: weights / q / v ----------------
            with tc.tile_pool(name="setsb", bufs=1) as ssb:
                w_nat = [
                    ssb.tile([128, 2 * H], f32, tag="wn", name=f"wn{i}")
                    for i in range(2)
                ]
                for eh in range(2):
                    rw = nc.sync.dma_start(
                        out=w_nat[eh], in_=wat[eh * 128 : (eh + 1) * 128, :]
                    )
                    setup_read_names.append(rw.ins.name)
                b_attn_sb = ssb.tile([1, H], f32)
                rb = nc.sync.dma_start(out=b_attn_sb, in_=bat)
                setup_read_names.append(rb.ins.name)
                v_sb = ssb.tile([1, H], f32)
                rv = nc.sync.dma_start(out=v_sb, in_=wv)
                setup_read_names.append(rv.ins.name)
                h_nat = ssb.tile([BL, H], f32)
                rh = nc.sync.dma_start(out=h_nat, in_=hid)
                setup_read_names.append(rh.ins.name)
                ones4 = ssb.tile([1, BL], f32)
                nc.vector.memset(ones4, 1.0)

                wht = [
                    ssb.tile([128, E], f32, tag=f"wht{i}", name=f"wht{i}")
                    for i in range(2)
                ]
                for eh in range(2):
                    for cblk in range(4):  # column blocks of W_attn
                        pt = pps.tile([128, 128], f32, tag="s", bufs=2, name="pt_w")
                        nc.tensor.transpose(
                            pt, w_nat[eh][:, cblk * 128 : (cblk + 1) * 128], ident
                        )
                        if cblk < 2:  # Wh columns
                            nc.scalar.copy(
                                out=wht[cblk][:, eh * 128 : (eh + 1) * 128], in_=pt
                            )
                        else:  # We columns -> fp8, pre-scaled
                            nc.scalar.activation(
                                out=wet8[:, cblk - 2, eh * 128 : (eh + 1) * 128],
                                in_=pt,
                                func=AF.Copy,
                                scale=WSCALE,
                            )

                ht = [
                    ssb.tile([128, BL], f32, tag=f"ht{i}", name=f"ht{i}")
                    for i in range(2)
                ]
                for hh in range(2):
                    pt = pps.tile([128, 128], f32, tag="s", bufs=2, name="pt_h")
                    nc.tensor.transpose(
                        pt[:, :BL], h_nat[:, hh * 128 : (hh + 1) * 128], ident[:BL, :BL]
                    )
                    nc.scalar.copy(out=ht[hh], in_=pt[:, :BL])

                for eh in range(2):
                    pt = pps.tile([128, 128], f32, tag="s", bufs=2, name="pt_v")
                    nc.tensor.transpose(
                        pt[:, :1], v_sb[:, eh * 128 : (eh + 1) * 128], ident[:1, :1]
                    )
                    nc.scalar.copy(out=vt16[eh], in_=pt[:, :1])

                # qb[eh][e, b] = sum_h WhT[h, e] * hT[h, b] + b_attn[e]
                for eh in range(2):
                    pq = pps.tile([128, 128], f32, tag="s", bufs=2, name="pt_q")
                    for hh in range(2):
                        nc.tensor.matmul(
                            pq[:, :BL],
                            wht[hh][:, eh * 128 : (eh + 1) * 128],
                            ht[hh],
                            start=(hh == 0),
                            stop=False,
                        )
                    nc.tensor.matmul(
                        pq[:, :BL],
                        b_attn_sb[:, eh * 128 : (eh + 1) * 128],
                        ones4,
                        start=False,
                        stop=True,
                    )
                    nc.scalar.copy(out=qb[eh], in_=pq[:, :BL])

            # ---------------- main loop ----------------
            pctx = [
                ppc.tile([2, 512], f32, tag=f"ctx{h}", name=f"pctx{h}")
                for h in range(2)
            ]

            def ctx_group(g):
                for half in range(2):
                    for jl in range(4):
                        n = g * 4 + jl
                        nc.tensor.matmul(
                            pctx[half],
                            u_g[g][:, 2 * half : 2 * half + 2, jl],
                            x_res[g][:, jl, half * 512 : (half + 1) * 512],
                            start=(n == 0),
                            stop=(n == NG * 4 - 1),
                        )

            for g in range(NG):
                xv = x_res[g].rearrange("p i (b hh c) -> p i b hh c", b=4, c=128)
                # --- PE transposes: 4 stages of (hh, b-pair), each one PSUM
                # bank of 8 [128,128] bf16 tiles, drained by DVE/ACT
                # copy-casts into the fp8 X^T.
                for bp in range(2):
                    for hh in range(2):
                        stg = ppt.tile([128, 8, 128], bf16, tag="pt", name="ptstg")
                        for bi in range(2):
                            for i in range(4):
                                nc.tensor.transpose(
                                    stg[:, bi * 4 + i, :],
                                    xv[:, i, bp * 2 + bi, hh, :],
                                    ident16,
                                )
                        dst = xt8[g][:, bp * 2 : bp * 2 + 2, hh]
                        src = stg.rearrange("p (b i) c -> p b (i c)", b=2)
                        if hh == 0:
                            nc.vector.tensor_copy(out=dst, in_=src)
                        else:
                            nc.scalar.copy(out=dst, in_=src)

                # context for the previous group: its data is ready, so it
                # fills the PE while this group's copies drain.
                if g >= 1:
                    ctx_group(g - 1)

                strip = pps.tile([97, 512], f32, tag="s", name="strip")
                for eh in range(2):
                    th_eh = []
                    for b in range(BL):
                        pe_t = ppe.tile([128, 512], f32, tag="e", name="pe")
                        nc.tensor.matmul(
                            pe_t,
                            wet8[:, :, eh * 128 : (eh + 1) * 128],
                            xt8[g][:, b],
                            start=True,
                            stop=True,
                            perf_mode=DR,
                        )
                        th = thp.tile([128, 512], bf16, tag="th", name="th")
                        nc.scalar.activation(
                            out=th,
                            in_=pe_t,
                            func=AF.Tanh,
                            bias=qb[eh][:, b : b + 1],
                            scale=1.0 / WSCALE,
                        )
                        th_eh.append(th)
                    for b in range(BL):
                        nc.tensor.matmul(
                            strip[32 * b : 32 * b + 1, :],
                            vt16[eh],
                            th_eh[b],
                            start=(eh == 0),
                            stop=(eh == 1),
                            tile_position=(0, 32 * b),
                        )

                nc.scalar.activation(
                    out=st_g[g],
                    in_=strip,
                    func=AF.Exp,
                    accum_out=acc_all[:, g : g + 1],
                )
                for c in range(4):
                    pt = pps.tile([128, 256], bf16, tag="s", bufs=2, name="pt_u")
                    nc.tensor.transpose(
                        pt[:, :97],
                        st_g[g][:, c * 128 : (c + 1) * 128],
                        ident16[:97, :97],
                    )
                    nc.vector.tensor_copy(
                        out=u_g[g][:, :, c],
                        in_=pt.rearrange("p (a r) -> p a r", r=32)[:, :4, 0],
                    )

            # denominators are ready right after group 7's exp; ship them
            # first so their HBM-write receipt hides under the context tail
            accs = wp.tile([97, 1], f32)
            nc.vector.reduce_sum(out=accs, in_=acc_all, axis=mybir.AxisListType.X)
            nc.sync.dma_start(out=den_t.ap(), in_=accs)

            g = NG - 1
            for half in range(2):
                for jl in range(4):
                    n = g * 4 + jl
                    nc.tensor.matmul(
                        pctx[half],
                        u_g[g][:, 2 * half : 2 * half + 2, jl],
                        x_res[g][:, jl, half * 512 : (half + 1) * 512],
                        start=(n == 0),
                        stop=(n == NG * 4 - 1),
                    )
                csb = wp.tile([2, 512], f32, tag="csb", name=f"csb{half}")
                nc.scalar.copy(out=csb, in_=pctx[half])
                nc.sync.dma_start(out=ctxu_t.ap()[half], in_=csb)

    _dma_surgery(nc, mybir, read_names, setup_read_names, verbose=verbose)
    _split_multiwait(nc, mybir)
    return nc


def kernel(**inputs):
    from concourse.bass_utils import run_bass_kernel_spmd

    hidden = np.asarray(inputs["hidden"], dtype=np.float32)
    enc = np.asarray(inputs["encoder_outputs"], dtype=np.float32)
    w_attn = np.ascontiguousarray(np.asarray(inputs["W_attn"], dtype=np.float32))
    b_attn = np.ascontiguousarray(np.asarray(inputs["b_attn"], dtype=np.float32))
    w_v = np.ascontiguousarray(np.asarray(inputs["W_v"], dtype=np.float32))

    if "nc" not in _CACHE:
        _CACHE["nc"] = _build()
    nc = _CACHE["nc"]

    in_maps = []
    for c in range(NCORES):
        sl = slice(c * BL, (c + 1) * BL)
        in_maps.append(
            {
                "hidden": np.ascontiguousarray(hidden[sl]),
                "enc": np.ascontiguousarray(enc[:, sl, :]),
                "w_attn": w_attn,
                "b_attn": b_attn,
                "w_v": w_v,
            }
        )

    trace = bool(_CACHE.get("trace", False))
    res = run_bass_kernel_spmd(nc, in_maps, core_ids=list(range(NCORES)), trace=trace)
    _CACHE["last_results"] = res

    out = np.empty((1, B, H), dtype=np.float32)
    for c in range(NCORES):
        ctxu = res.results[c]["ctxu"]  # [2, 2, 512]
        den = res.results[c]["den"]    # [97, 1]
        for b in range(BL):
            half, row = b // 2, b % 2
            vals = ctxu[half, row, row * 256 : row * 256 + 256]
            out[0, c * BL + b] = vals / den[32 * b, 0]
    return out
